# revision 17
# baseline (speedup 1.0000x reference)
"""Trainium2 Bass kernel for 3-layer GINE-style GNN message passing
(DenseNet concat + final linear head), distributed over 8 NeuronCores.

Sharding: nodes are split into 8 contiguous blocks of N/8; each core owns its
block's rows. Edges are assigned to the core owning their destination node,
sorted by destination, and padded so every core runs an identical instruction
stream (single SPMD IR, per-core input data).

Per layer, each core gathers source-node features from a replicated "z table"
in DRAM (T1 = raw normalized x; T2/T3 = leaky(z) of all prior column groups,
valid because the reference's BatchNorm gamma/beta are structurally 1/0).
Edge messages m = relu(z[src] + edge_attr @ We.T + be) are aggregated per
128-node window with one-hot matmuls (S[e, n] = (dstoff[e] == n)), the node
MLP runs on transposed tiles (features on partitions) in window pairs so the
moving free dim is 256 and float32r matmuls run at full rate, and BatchNorm
statistics cross cores via tiny AllReduces.  New column groups are AllGathered
into the next layer's table.  The final head accumulates h'_g @ Wf_g per
window as each group is produced, so raw h is never materialized.
"""

import numpy as np
import ml_dtypes
from contextlib import ExitStack

import concourse.bass as bass
import concourse.tile as tile
from concourse import bacc, mybir
from concourse.bass_utils import run_bass_kernel_spmd
from concourse.library_config import mlp as _mlp_lib
from concourse.masks import make_identity

P = 128
NCORES = 8
HALF = 32768          # int16 index split for dma_gather
BN_EPS = 1e-5
SLOPE = 0.01
D_NODE = 128
D_EDGE = 16
D_CONV = 128
N_LAYERS = 3

f32 = mybir.dt.float32
f32r = mybir.dt.float32r
bf16 = mybir.dt.bfloat16
i16 = mybir.dt.int16

BF = ml_dtypes.bfloat16


# ----------------------------------------------------------------------------
# host-side preprocessing
# ----------------------------------------------------------------------------

def _preprocess_edges(edge_index, edge_attr, n_nodes, nloc):
    """Partition edges by destination owner, sort by destination, split each
    128-node window's edges into low-src (< HALF) and high-src sections,
    pad every (window, section) to a whole number of 128-edge chunks common
    to all cores.  Pad edges gather row 0 and carry dstoff -1 so their
    one-hot row is all zeros (no contribution).
    """
    src = np.asarray(edge_index[0], dtype=np.int64)
    dst = np.asarray(edge_index[1], dtype=np.int64)
    ea = np.asarray(edge_attr, dtype=np.float32)
    nw = (nloc + P - 1) // P

    per = [[[None, None] for _ in range(nw)] for _ in range(NCORES)]
    owner = dst // nloc
    for r in range(NCORES):
        msel = owner == r
        s_r = src[msel]
        d_r = dst[msel] - r * nloc
        order = np.argsort(d_r, kind="stable")
        s_r, d_r = s_r[order], d_r[order]
        ea_r = ea[msel][order]
        w_r = d_r // P
        lowsel = s_r < HALF
        for w in range(nw):
            wsel = w_r == w
            for si, ss in enumerate([wsel & lowsel, wsel & ~lowsel]):
                per[r][w][si] = (s_r[ss], d_r[ss] - w * P, ea_r[ss])

    CWL, CWH = [], []
    for w in range(nw):
        cl = max((len(per[r][w][0][0]) + P - 1) // P for r in range(NCORES))
        ch = max((len(per[r][w][1][0]) + P - 1) // P for r in range(NCORES))
        if cl + ch == 0:
            cl = 1
        CWL.append(cl)
        CWH.append(ch)
    nchunk = sum(CWL) + sum(CWH)
    slots = nchunk * P

    idx16 = np.zeros((NCORES, slots), dtype=np.int16)
    dstoff = np.full((NCORES, slots), -1.0, dtype=np.float32)
    attrT = np.zeros((NCORES, 17, slots), dtype=BF)
    attrT[:, 16, :] = BF(1.0)

    for r in range(NCORES):
        s0 = 0
        for w in range(nw):
            for si, cw in ((0, CWL[w]), (1, CWH[w])):
                s_e, doff, ea_e = per[r][w][si]
                k = len(s_e)
                sl = slice(s0, s0 + k)
                idx16[r, sl] = (s_e - HALF * si).astype(np.int16)
                dstoff[r, sl] = doff.astype(np.float32)
                attrT[r, :16, sl] = ea_e.T.astype(BF)
                s0 += cw * P

    idx_wrap = np.zeros((NCORES, P, slots // 16), dtype=np.int16)
    a = idx16.reshape(NCORES, slots // 16, 16).transpose(0, 2, 1)
    for g in range(8):
        idx_wrap[:, g * 16:(g + 1) * 16, :] = a
    dstoff_dev = dstoff.reshape(NCORES, nchunk, P).transpose(0, 2, 1).copy()
    return idx_wrap, dstoff_dev, attrT, CWL, CWH


# ----------------------------------------------------------------------------
# device kernel builder
# ----------------------------------------------------------------------------

def _build(meta):
    n = meta["n"]
    nloc = meta["nloc"]
    nw = meta["nw"]
    CWL, CWH = meta["CWL"], meta["CWH"]
    nchunk = sum(CWL) + sum(CWH)
    slots = nchunk * P
    eps_val = meta["eps"]
    bf_val = meta["bf"]
    table_bf16 = meta["table_bf16"]
    tdt = bf16 if table_bf16 else f32

    csz = [l * D_CONV + D_NODE for l in range(N_LAYERS)]      # 128, 256, 384
    ctot = csz[-1] + D_CONV                                    # 512

    wins = [(w, min(P, nloc - w * P)) for w in range(nw)]
    pairs = []
    w = 0
    while w < nw:
        if w + 1 < nw:
            pairs.append((w, w + 1))
            w += 2
        else:
            pairs.append((w, None))
            w += 1
    npairs = len(pairs)

    ci_base = [0] * nw
    acc = 0
    for w in range(nw):
        ci_base[w] = acc
        acc += CWL[w] + CWH[w]

    nc = bacc.Bacc("TRN2", target_bir_lowering=False, debug=False,
                   num_devices=NCORES)

    def din(name, shape, dt):
        return nc.dram_tensor(name, shape, dt, kind="ExternalInput").ap()

    def dint(name, shape, dt):
        return nc.dram_tensor(name, shape, dt).ap()

    x_loc = din("x_loc", [nloc, D_NODE], f32)
    idx16_in = din("idx16", [P, slots // 16], i16)
    dstoff_in = din("dstoff", [P, nchunk], f32)
    attrT_in = din("attrT", [17, slots], bf16)
    iota_in = din("iota", [P, P], f32)
    wet_in = [din(f"wet{l}", [17, csz[l]], bf16) for l in range(N_LAYERS)]
    w1t_in = [din(f"w1t{l}", [csz[l], D_CONV], f32) for l in range(N_LAYERS)]
    w2t_in = [din(f"w2t{l}", [D_CONV, D_CONV], f32) for l in range(N_LAYERS)]
    g2b2_in = [din(f"g2b2{l}", [P, 2], f32) for l in range(N_LAYERS)]
    b2c_in = [din(f"b2c{l}", [P, 1], f32) for l in range(N_LAYERS)]
    wf_in = din("wf", [ctot, 1], f32)

    out_grid = nc.dram_tensor("out_grid", [P, nw], f32,
                              kind="ExternalOutput").ap()

    T1 = dint("T1", [n, D_NODE], f32)
    T23 = [None, dint("T2", [n, csz[1]], tdt), dint("T3", [n, csz[2]], tdt)]
    PAY1 = dint("PAY1", [nloc, D_NODE], f32)
    PAY23 = [None, dint("PAY2", [nloc, csz[1]], tdt),
             dint("PAY3", [nloc, csz[2]], tdt)]
    ar_in = [dint(f"arin{i}", [P, 2], f32) for i in range(6)]
    ar_out = [dint(f"arout{i}", [P, 2], f32) for i in range(6)]
    rg = [list(range(NCORES))]

    AX = mybir.AxisListType.X
    AF = mybir.ActivationFunctionType
    OP = mybir.AluOpType
    simc = meta.get("simc", False)

    with tile.TileContext(nc) as tc, ExitStack() as ctx:
        nc.gpsimd.load_library(_mlp_lib)

        consts = ctx.enter_context(tc.tile_pool(name="consts", bufs=1))
        stores = ctx.enter_context(tc.tile_pool(name="stores", bufs=2))
        sbw = ctx.enter_context(tc.tile_pool(name="sbw", bufs=3))
        sbsm = ctx.enter_context(tc.tile_pool(name="sbsm", bufs=4))
        ps_e = ctx.enter_context(tc.tile_pool(name="ps_e", bufs=2, space="PSUM"))
        ps_a = ctx.enter_context(tc.tile_pool(name="ps_a", bufs=1, space="PSUM"))
        ps_h = ctx.enter_context(tc.tile_pool(name="ps_h", bufs=1, space="PSUM"))
        ps_u = ctx.enter_context(tc.tile_pool(name="ps_u", bufs=1, space="PSUM"))
        ps_x = ctx.enter_context(tc.tile_pool(name="ps_x", bufs=1, space="PSUM"))

        # ---- resident constants ----
        iota_t = consts.tile([P, P], f32, tag="iota_t")
        nc.sync.dma_start(out=iota_t[:], in_=iota_in[:])
        ident = consts.tile([P, P], f32, tag="ident")
        make_identity(nc, ident[:])
        ident_r = consts.tile([P, P], f32r, tag="ident_r")
        nc.vector.tensor_copy(out=ident_r[:], in_=ident[:])
        ident_b = consts.tile([P, P], bf16, tag="ident_b")
        nc.vector.tensor_copy(out=ident_b[:], in_=ident[:])
        ident_tb = ident_b if table_bf16 else ident
        idx_t = consts.tile([P, slots // 16], i16, tag="idx_t")
        nc.sync.dma_start(out=idx_t[:], in_=idx16_in[:])
        dstoff_t = consts.tile([P, nchunk], f32, tag="dstoff_t")
        nc.sync.dma_start(out=dstoff_t[:], in_=dstoff_in[:])

        wet_t, w1t_t, w2t_t, g2b2_t, b2c_t = [], [], [], [], []
        for l in range(N_LAYERS):
            t = consts.tile([17, csz[l]], bf16, tag=f"wet{l}", name=f"wet_t{l}")
            nc.sync.dma_start(out=t[:], in_=wet_in[l][:])
            wet_t.append(t)
            w1chunks = []
            for j in range(csz[l] // P):
                tmp = consts.tile([P, D_CONV], f32, tag=f"w1s{l}_{j}",
                                  name=f"w1s{l}_{j}")
                nc.sync.dma_start(out=tmp[:],
                                  in_=w1t_in[l][j * P:(j + 1) * P, :])
                trj = consts.tile([P, D_CONV], f32r, tag=f"w1r{l}_{j}",
                                  name=f"w1r{l}_{j}")
                nc.vector.tensor_copy(out=trj[:], in_=tmp[:])
                w1chunks.append(trj)
            w1t_t.append(w1chunks)
            tmp2 = consts.tile([D_CONV, D_CONV], f32, tag=f"w2s{l}",
                               name=f"w2s{l}")
            nc.sync.dma_start(out=tmp2[:], in_=w2t_in[l][:])
            tr2 = consts.tile([D_CONV, D_CONV], f32r, tag=f"w2r{l}",
                              name=f"w2r{l}")
            nc.vector.tensor_copy(out=tr2[:], in_=tmp2[:])
            w2t_t.append(tr2)
            t3 = consts.tile([P, 2], f32, tag=f"g2b2{l}", name=f"g2b2t{l}")
            nc.sync.dma_start(out=t3[:], in_=g2b2_in[l][:])
            g2b2_t.append(t3)
            t4 = consts.tile([P, 1], f32, tag=f"b2c{l}", name=f"b2ct{l}")
            nc.sync.dma_start(out=t4[:], in_=b2c_in[l][:])
            b2c_t.append(t4)
        wf_t = []
        for g in range(ctot // P):
            tg = consts.tile([P, 1], f32, tag=f"wf{g}", name=f"wf_t{g}")
            nc.sync.dma_start(out=tg[:], in_=wf_in[g * P:(g + 1) * P, :])
            wf_t.append(tg)

        headacc = consts.tile([P, nw], f32, tag="headacc")
        nc.gpsimd.memset(headacc[:], 0.0)
        scr = consts.tile([P, 256], f32, tag="scr")
        stat_cols = consts.tile([P, 2 * npairs], f32, tag="stat_cols")
        stat_pack = consts.tile([P, 2], f32, tag="stat_pack")

        def stats_allreduce(idx):
            nc.vector.reduce_sum(out=stat_pack[:, 0:1],
                                 in_=stat_cols[:, 0:npairs], axis=AX)
            nc.vector.reduce_sum(out=stat_pack[:, 1:2],
                                 in_=stat_cols[:, npairs:2 * npairs], axis=AX)
            nc.sync.dma_start(out=ar_in[idx][:], in_=stat_pack[:])
            nc.gpsimd.collective_compute(
                "AllReduce", OP.add, replica_groups=rg,
                ins=[ar_in[idx][:]], outs=[ar_out[idx][:]])
            g = sbsm.tile([P, 2], f32, tag="gstat", name=f"gstat{idx}")
            nc.sync.dma_start(out=g[:], in_=ar_out[idx][:])
            return g

        def bn_coeffs(gsum, gamma=None, beta=None):
            """[P,2] global sums -> [P,4] tile; col2 = scale, col3 = bias."""
            c = sbsm.tile([P, 4], f32, tag="bnc", name=f"bnc{bn_coeffs.i}")
            bn_coeffs.i += 1
            nc.vector.tensor_scalar_mul(out=c[:, 0:1], in0=gsum[:, 0:1],
                                        scalar1=1.0 / n)
            nc.vector.tensor_scalar_mul(out=c[:, 1:2], in0=gsum[:, 1:2],
                                        scalar1=1.0 / n)
            nc.vector.tensor_tensor(out=scr[:, 0:1], in0=c[:, 0:1],
                                    in1=c[:, 0:1], op=OP.mult)
            nc.vector.tensor_tensor(out=c[:, 1:2], in0=c[:, 1:2],
                                    in1=scr[:, 0:1], op=OP.subtract)
            nc.vector.tensor_scalar_add(out=c[:, 1:2], in0=c[:, 1:2],
                                        scalar1=BN_EPS)
            nc.scalar.activation(out=c[:, 2:3], in_=c[:, 1:2], func=AF.Sqrt)
            nc.vector.reciprocal(out=c[:, 2:3], in_=c[:, 2:3])
            if gamma is not None:
                nc.vector.tensor_tensor(out=c[:, 2:3], in0=c[:, 2:3],
                                        in1=gamma, op=OP.mult)
            nc.vector.tensor_tensor(out=c[:, 3:4], in0=c[:, 0:1],
                                    in1=c[:, 2:3], op=OP.mult)
            if beta is not None:
                nc.vector.tensor_tensor(out=c[:, 3:4], in0=beta, in1=c[:, 3:4],
                                        op=OP.subtract)
            else:
                nc.vector.tensor_scalar_mul(out=c[:, 3:4], in0=c[:, 3:4],
                                            scalar1=-1.0)
            return c
        bn_coeffs.i = 0

        def pair_cols(pi):
            w0, w1 = pairs[pi]
            return sum(wins[w][1] for w in ([w0] if w1 is None else [w0, w1]))

        def act_leaky(out, in_, scale, bias, cols):
            """out = leaky_relu(in_ * scale + bias); sim lacks Lrelu."""
            if not simc:
                nc.scalar.activation(out=out, in_=in_, func=AF.Lrelu,
                                     bias=bias, scale=scale, alpha=SLOPE)
                return
            aff = sbw.tile([P, 256], f32, tag="lkA")
            nc.scalar.activation(out=aff[:, :cols], in_=in_, func=AF.Identity,
                                 bias=bias, scale=scale)
            sc = sbw.tile([P, 256], f32, tag="lkB")
            nc.vector.tensor_scalar_mul(out=sc[:, :cols], in0=aff[:, :cols],
                                        scalar1=SLOPE)
            nc.vector.tensor_tensor(out=out, in0=aff[:, :cols],
                                    in1=sc[:, :cols], op=OP.max)

        # ================= PREP: x stats, z0 tables, head(x) =================
        xT_store = stores.tile([P, nw * P], f32, tag="big", name="xT_store")
        for w, nwn in wins:
            xw = sbw.tile([P, D_NODE], f32, tag="xw")
            nc.sync.dma_start(out=xw[:nwn, :], in_=x_loc[w * P:w * P + nwn, :])
            tp = ps_x.tile([P, P], f32, tag="aux", name="tp_x")
            nc.tensor.transpose(out=tp[:, :nwn], in_=xw[:nwn, :],
                                identity=ident[:nwn, :nwn])
            nc.vector.tensor_copy(out=xT_store[:, w * P:w * P + nwn],
                                  in_=tp[:, :nwn])
        for pi, (w0, w1) in enumerate(pairs):
            pc = pair_cols(pi)
            sl = xT_store[:, w0 * P:w0 * P + pc]
            nc.vector.reduce_sum(out=stat_cols[:, pi:pi + 1], in_=sl, axis=AX)
            nc.vector.tensor_tensor(out=scr[:, :pc], in0=sl, in1=sl,
                                    op=OP.mult)
            nc.vector.reduce_sum(out=stat_cols[:, npairs + pi:npairs + pi + 1],
                                 in_=scr[:, :pc], axis=AX)
        c0 = bn_coeffs(stats_allreduce(0))

        for w, nwn in wins:
            xT = xT_store[:, w * P:w * P + nwn]
            z0T = sbw.tile([P, P], f32, tag="z0T")
            nc.scalar.activation(out=z0T[:, :nwn], in_=xT, func=AF.Identity,
                                 bias=c0[:, 3:4], scale=c0[:, 2:3])
            tp = ps_x.tile([P, P], f32, tag="aux", name="tp_z")
            nc.tensor.transpose(out=tp[:nwn, :], in_=z0T[:, :nwn],
                                identity=ident[:])
            rows = sbw.tile([P, P], f32, tag="rows")
            nc.vector.tensor_copy(out=rows[:nwn, :], in_=tp[:nwn, :])
            nc.sync.dma_start(out=PAY1[w * P:w * P + nwn, :], in_=rows[:nwn, :])
            lzT = sbw.tile([P, P], tdt, tag="lzT")
            act_leaky(lzT[:, :nwn], xT, c0[:, 2:3], c0[:, 3:4], nwn)
            tpb = ps_x.tile([P, P], tdt, tag="aux", name="tp_lz")
            nc.tensor.transpose(out=tpb[:nwn, :], in_=lzT[:, :nwn],
                                identity=ident_tb[:])
            rowsb = sbw.tile([P, P], tdt, tag="rowsb")
            nc.vector.tensor_copy(out=rowsb[:nwn, :], in_=tpb[:nwn, :])
            nc.sync.dma_start(out=PAY23[1][w * P:w * P + nwn, 0:D_NODE],
                              in_=rowsb[:nwn, :])
            nc.sync.dma_start(out=PAY23[2][w * P:w * P + nwn, 0:D_NODE],
                              in_=rowsb[:nwn, :])
            hp_ps = ps_u.tile([P, 1], f32, tag="uT", name="head_x")
            nc.tensor.matmul(out=hp_ps[:nwn, :], lhsT=xT, rhs=wf_t[0][:],
                             start=True, stop=True)
            nc.vector.tensor_tensor(out=headacc[:nwn, w:w + 1],
                                    in0=headacc[:nwn, w:w + 1],
                                    in1=hp_ps[:nwn, :], op=OP.add)
        nc.gpsimd.collective_compute(
            "AllGather", OP.bypass, replica_groups=rg,
            ins=[PAY1[:]], outs=[T1[:]])

        stop = meta.get("stop_after", "")

        # ================= layers =================
        for l in range(N_LAYERS):
            if stop == "prep" or stop == f"layer{l - 1}":
                break
            c = csz[l]
            ncc = c // P
            table = T1 if l == 0 else T23[l]
            pay = PAY1 if l == 0 else PAY23[l]
            gdt = f32 if l == 0 else tdt

            u_store = stores.tile([P, nw * P], f32, tag="big",
                                  name=f"u_store{l}")
            for pi, (w0, w1) in enumerate(pairs):
                pw = [w0] if w1 is None else [w0, w1]
                pc = pair_cols(pi)
                hpT_ps = [ps_h.tile([P, 256], f32r, tag=f"hpT{j}",
                                    name=f"hpT{j}_{l}_{pi}")
                          for j in range(ncc)]
                for wi, w in enumerate(pw):
                    nwn = wins[w][1]
                    cw_l, cw_h = CWL[w], CWH[w]
                    cw = cw_l + cw_h
                    cb = ci_base[w]
                    xg = sbw.tile([P, cw * c], gdt, tag="xg")
                    secs = []
                    if cw_l:
                        secs.append((0, 0, cw_l))
                    if cw_h:
                        secs.append((1, cw_l, cw_h))
                    for si, coff, cws in secs:
                        tview = table[:min(HALF, n)] if si == 0 else table[HALF:]
                        c0c = cb + coff
                        nc.gpsimd.dma_gather(
                            xg[:, coff * c:(coff + cws) * c]
                            .rearrange("p (k d) -> p k d", d=c),
                            tview, idx_t[:, c0c * 8:(c0c + cws) * 8],
                            cws * P, cws * P, c,
                            single_packet=(cws * P <= 1024))
                    attr_w = sbw.tile([17, cw * P], bf16, tag="attr")
                    nc.sync.dma_start(out=attr_w[:],
                                      in_=attrT_in[:, cb * P:(cb + cw) * P])
                    agg_ps = ps_a.tile([P, c], f32, tag="agg",
                                       name=f"agg{l}_{w}")
                    for t in range(cw):
                        ci = cb + t
                        e_ps = ps_e.tile([P, c], f32, tag="e")
                        nc.tensor.matmul(out=e_ps[:],
                                         lhsT=attr_w[:, t * P:(t + 1) * P],
                                         rhs=wet_t[l][:], start=True, stop=True)
                        s_t = sbsm.tile([P, P], bf16, tag="s")
                        nc.vector.tensor_tensor(
                            out=s_t[:],
                            in0=dstoff_t[:, ci:ci + 1].to_broadcast([P, P]),
                            in1=iota_t[:], op=OP.is_equal)
                        msum = sbsm.tile([P, c], f32, tag="msum")
                        nc.vector.tensor_tensor(out=msum[:],
                                                in0=xg[:, t * c:(t + 1) * c],
                                                in1=e_ps[:], op=OP.add)
                        m_t = sbsm.tile([P, c], bf16, tag="m")
                        nc.scalar.activation(out=m_t[:], in_=msum[:],
                                             func=AF.Relu)
                        nc.tensor.matmul(out=agg_ps[:], lhsT=s_t[:], rhs=m_t[:],
                                         start=(t == 0), stop=(t == cw - 1))
                    zl = sbw.tile([P, c], gdt, tag="zl")
                    nc.sync.dma_start(out=zl[:nwn, :],
                                      in_=pay[w * P:w * P + nwn, :])
                    hp = sbw.tile([P, c], f32r, tag="hp")
                    if eps_val[l] != 0.0:
                        zs = sbw.tile([P, c], f32, tag="zs")
                        nc.vector.tensor_scalar_mul(out=zs[:nwn, :],
                                                    in0=zl[:nwn, :],
                                                    scalar1=1.0 + eps_val[l])
                        nc.vector.tensor_tensor(out=hp[:nwn, :],
                                                in0=agg_ps[:nwn, :],
                                                in1=zs[:nwn, :], op=OP.add)
                    else:
                        nc.vector.tensor_tensor(out=hp[:nwn, :],
                                                in0=agg_ps[:nwn, :],
                                                in1=zl[:nwn, :], op=OP.add)
                    for j in range(ncc):
                        nc.tensor.transpose(
                            out=hpT_ps[j][:, wi * P:wi * P + nwn],
                            in_=hp[:nwn, j * P:(j + 1) * P],
                            identity=ident_r[:nwn, :nwn])
                uT_ps = ps_u.tile([P, 256], f32, tag="uT", name=f"uT{l}_{pi}")
                for j in range(ncc):
                    hsb = sbw.tile([P, 256], f32r, tag=f"hpsb{j}")
                    nc.vector.tensor_copy(out=hsb[:, :pc],
                                          in_=hpT_ps[j][:, :pc])
                    nc.tensor.matmul(out=uT_ps[:, :pc],
                                     lhsT=w1t_t[l][j][:],
                                     rhs=hsb[:, :pc],
                                     start=(j == 0), stop=(j == ncc - 1))
                usl = u_store[:, w0 * P:w0 * P + pc]
                nc.vector.tensor_copy(out=usl, in_=uT_ps[:, :pc])
                nc.vector.reduce_sum(out=stat_cols[:, pi:pi + 1], in_=usl,
                                     axis=AX)
                nc.vector.tensor_tensor(out=scr[:, :pc], in0=usl, in1=usl,
                                        op=OP.mult)
                nc.vector.reduce_sum(
                    out=stat_cols[:, npairs + pi:npairs + pi + 1],
                    in_=scr[:, :pc], axis=AX)
            if stop == f"pairs{l}":
                break
            ca = bn_coeffs(stats_allreduce(1 + 2 * l),
                           gamma=g2b2_t[l][:, 0:1], beta=g2b2_t[l][:, 1:2])

            hT_store = stores.tile([P, nw * P], f32, tag="big",
                                   name=f"hT_store{l}")
            for pi, (w0, w1) in enumerate(pairs):
                pw = [w0] if w1 is None else [w0, w1]
                pc = pair_cols(pi)
                usl = u_store[:, w0 * P:w0 * P + pc]
                vT = sbw.tile([P, 256], f32r, tag="vT")
                act_leaky(vT[:, :pc], usl, ca[:, 2:3], ca[:, 3:4], pc)
                hT_ps = ps_u.tile([P, 256], f32, tag="uT", name=f"hT{l}_{pi}")
                nc.tensor.matmul(out=hT_ps[:, :pc], lhsT=w2t_t[l][:],
                                 rhs=vT[:, :pc], start=True, stop=True)
                hsl = hT_store[:, w0 * P:w0 * P + pc]
                nc.scalar.activation(out=hsl, in_=hT_ps[:, :pc],
                                     func=AF.Identity, bias=b2c_t[l][:, 0:1])
                for w in pw:
                    nwn = wins[w][1]
                    hp_ps = ps_u.tile([P, 1], f32, tag="uT",
                                      name=f"head{l}_{w}")
                    nc.tensor.matmul(out=hp_ps[:nwn, :],
                                     lhsT=hT_store[:, w * P:w * P + nwn],
                                     rhs=wf_t[l + 1][:],
                                     start=True, stop=True)
                    nc.vector.tensor_tensor(out=headacc[:nwn, w:w + 1],
                                            in0=headacc[:nwn, w:w + 1],
                                            in1=hp_ps[:nwn, :], op=OP.add)
                if l < N_LAYERS - 1:
                    nc.vector.reduce_sum(out=stat_cols[:, pi:pi + 1], in_=hsl,
                                         axis=AX)
                    nc.vector.tensor_tensor(out=scr[:, :pc], in0=hsl, in1=hsl,
                                            op=OP.mult)
                    nc.vector.reduce_sum(
                        out=stat_cols[:, npairs + pi:npairs + pi + 1],
                        in_=scr[:, :pc], axis=AX)
            if stop == f"post{l}":
                break
            if l < N_LAYERS - 1:
                cb_ = bn_coeffs(stats_allreduce(2 + 2 * l))
                gcol = D_NODE * (l + 1)
                for w, nwn in wins:
                    hsl = hT_store[:, w * P:w * P + nwn]
                    lzT = sbw.tile([P, P], tdt, tag="lzT")
                    act_leaky(lzT[:, :nwn], hsl, cb_[:, 2:3], cb_[:, 3:4], nwn)
                    tpb = ps_x.tile([P, P], tdt, tag="aux", name=f"tpb{l}_{w}")
                    nc.tensor.transpose(out=tpb[:nwn, :], in_=lzT[:, :nwn],
                                        identity=ident_tb[:])
                    rowsb = sbw.tile([P, P], tdt, tag="rowsb")
                    nc.vector.tensor_copy(out=rowsb[:nwn, :], in_=tpb[:nwn, :])
                    for l2 in range(l + 1, N_LAYERS):
                        nc.sync.dma_start(
                            out=PAY23[l2][w * P:w * P + nwn,
                                          gcol:gcol + D_CONV],
                            in_=rowsb[:nwn, :])
                nc.gpsimd.collective_compute(
                    "AllGather", OP.bypass, replica_groups=rg,
                    ins=[PAY23[l + 1][:]], outs=[T23[l + 1][:]])

        # ================= output =================
        out_t = sbw.tile([P, nw], f32, tag="outt")
        nc.vector.tensor_scalar_add(out=out_t[:], in0=headacc[:],
                                    scalar1=float(bf_val))
        nc.sync.dma_start(out=out_grid[:], in_=out_t[:])

    nc.compile()
    return nc


# ----------------------------------------------------------------------------
# top-level entry
# ----------------------------------------------------------------------------

def _kernel_impl(x, edge_attr, edge_index, params, table_bf16=True):
    x = np.asarray(x, dtype=np.float32)
    edge_attr = np.asarray(edge_attr, dtype=np.float32)
    n, dnode = x.shape
    assert dnode == D_NODE
    assert n % NCORES == 0
    nloc = n // NCORES
    nw = (nloc + P - 1) // P

    layers = params["layers"]
    assert len(layers) == N_LAYERS
    for p in layers:
        assert np.allclose(np.asarray(p["bn_g"]), 1.0), "bn_g != 1 unsupported"
        assert np.allclose(np.asarray(p["bn_b"]), 0.0), "bn_b != 0 unsupported"

    idx_wrap, dstoff_dev, attrT, CWL, CWH = _preprocess_edges(
        edge_index, edge_attr, n, nloc)

    meta = {
        "n": n, "nloc": nloc, "nw": nw, "CWL": CWL, "CWH": CWH,
        "eps": [float(np.asarray(p["eps"])) for p in layers],
        "bf": float(np.asarray(params["bf"]).reshape(-1)[0]),
        "table_bf16": table_bf16,
    }
    nc = _build(meta)

    iota_np = np.tile(np.arange(P, dtype=np.float32), (P, 1))
    in_shared = {
        "iota": iota_np,
        "wf": np.asarray(params["Wf"], dtype=np.float32).reshape(-1, 1),
    }
    for l, p in enumerate(layers):
        we = np.asarray(p["We"], dtype=np.float32)
        be = np.asarray(p["be"], dtype=np.float32)
        in_shared[f"wet{l}"] = np.concatenate(
            [we.T, be[None, :]], axis=0).astype(BF)
        in_shared[f"w1t{l}"] = np.asarray(p["W1"], dtype=np.float32).T.copy()
        in_shared[f"w2t{l}"] = np.asarray(p["W2"], dtype=np.float32).T.copy()
        in_shared[f"g2b2{l}"] = np.stack(
            [np.asarray(p["g2"], dtype=np.float32),
             np.asarray(p["b2"], dtype=np.float32)], axis=1)
        in_shared[f"b2c{l}"] = np.asarray(
            p["b2c"], dtype=np.float32).reshape(-1, 1)

    in_maps = []
    for r in range(NCORES):
        m = dict(in_shared)
        m["x_loc"] = x[r * nloc:(r + 1) * nloc]
        m["idx16"] = idx_wrap[r]
        m["dstoff"] = dstoff_dev[r]
        m["attrT"] = attrT[r]
        in_maps.append(m)

    _LAST["nc"] = nc
    _LAST["in_maps"] = in_maps
    res = run_bass_kernel_spmd(nc, in_maps, list(range(NCORES)))
    outs = []
    for r in range(NCORES):
        grid = np.asarray(res.results[r]["out_grid"])
        outs.append(grid.T.reshape(-1)[:nloc])
    return np.concatenate(outs, axis=0)


_LAST = {}


def kernel(x, edge_attr, edge_index, params):
    return _kernel_impl(x, edge_attr, edge_index, params)
